# revision 26
# baseline (speedup 1.0000x reference)
"""LinearAttention kernel for one TRN2 chip (8 NeuronCores), Bass/Tile.

Math (per batch b):
  qkv = x @ w_qkv.T ; q,k,v split, per-head [n, 64]
  k_s = softmax(k, axis=-1)              (over dh, per token/head)
  context_h = k_s^T @ v                  [64, 64]
  out_h = q_h @ context_h ; y = out @ w_out.T + b

Restructured as:
  CT_h = (v/s)^T-style partial:  CT[e,d] = sum_n v[n,e]/s[n,h] * exp(k[n,d])
  G_h  = context_h @ w_out_h^T   -> G [inner=512, 1024] block rows
  y    = q @ G + b               (single K=512 matmul)

Sharding: 8 shards = (batch, half-sequence); each core computes its
2048 tokens end-to-end; only the tiny per-batch context (128 KiB) is
all-reduced between the two cores sharing a batch.

Device layouts avoid all transposes on-chip: the host feeds x^T, w^T.
Compute dtype bf16 (fp32 PSUM accumulation), output fp32.
"""

import contextlib
import ctypes
import os
import sys
import types

import numpy as np
import ml_dtypes

# ---------------------------------------------------------------------------
# Compat shim 1: the walrus in this image supports only ONE semaphore wait
# per instruction; split multi-wait instructions into prefix NoOps.
# ---------------------------------------------------------------------------
_MAX_WAITS = 1


def _legalize_bir(bir_bytes: bytes) -> bytes:
    import orjson

    bir = orjson.loads(bir_bytes)
    changed = False
    for fn in bir.get("functions", []):
        for blk in fn.get("blocks", []):
            new_insts = []
            for ins in blk.get("instructions", []):
                si = ins.get("sync_info") or {}
                waits = si.get("on_wait") or []
                if len(waits) > _MAX_WAITS:
                    changed = True
                    extra, keep = waits[:-_MAX_WAITS], waits[-_MAX_WAITS:]
                    for i in range(0, len(extra), _MAX_WAITS):
                        new_insts.append(
                            {
                                "name": f"{ins['name']}-ws{i}",
                                "opcode": "NoOp",
                                "engine": ins["engine"],
                                "ins": [],
                                "outs": [],
                                "sync_info": {
                                    "on_update": [],
                                    "on_wait": extra[i : i + _MAX_WAITS],
                                },
                            }
                        )
                    si["on_wait"] = keep
                new_insts.append(ins)
            blk["instructions"] = new_insts
    if not changed:
        return bir_bytes
    return orjson.dumps(bir)


_compile_patched = False


def _install_compile_patch():
    global _compile_patched
    if _compile_patched:
        return
    import concourse.bass2jax as bass2jax
    import concourse.bass_utils as bass_utils

    orig = bass2jax.compile_bir_kernel

    def compile_bir_kernel_legalized(bir_json, tmpdir, neff_name="file.neff"):
        return orig(_legalize_bir(bytes(bir_json)), tmpdir, neff_name=neff_name)

    bass2jax.compile_bir_kernel = compile_bir_kernel_legalized

    if os.environ.get("LINATTN_LDW_OPT"):
        orig_run = bass_utils.run_command

        def run_command_ldwopt(argv, **kw):
            argv = [
                "--enable-ldw-opt=true" if a == "--enable-ldw-opt=false" else a
                for a in argv
            ]
            return orig_run(argv, **kw)

        bass_utils.run_command = run_command_ldwopt
    _compile_patched = True


# ---------------------------------------------------------------------------
# Compat shim 2: NTFF profiling hook (only needed when BASS_TRACE is set).
# ---------------------------------------------------------------------------
def _install_ntff_hook():
    import antenv

    if "antenv.axon_hooks" in sys.modules:
        return
    so_path = "/opt/axon/libaxon_pjrt.so"

    def _mk(so_path):
        try:
            lib = ctypes.CDLL(so_path)
        except OSError:
            return None
        if not hasattr(lib, "axon_start_nrt_profile"):
            return None
        lib.axon_start_nrt_profile.argtypes = [
            ctypes.POINTER(ctypes.c_int64),
            ctypes.c_size_t,
        ]
        lib.axon_start_nrt_profile.restype = ctypes.c_int64
        lib.axon_stop_nrt_profile.argtypes = [ctypes.c_char_p]
        lib.axon_stop_nrt_profile.restype = ctypes.c_int64

        @contextlib.contextmanager
        def _hook(output_dir, device_ids):
            import jax

            jax.devices()
            if device_ids:
                ids = (ctypes.c_int64 * len(device_ids))(*device_ids)
                rc = lib.axon_start_nrt_profile(ids, len(device_ids))
            else:
                rc = lib.axon_start_nrt_profile(None, 0)
            if rc != 0:
                raise RuntimeError(f"axon_start_nrt_profile rc={rc}")
            try:
                yield
            finally:
                n = lib.axon_stop_nrt_profile(str(output_dir).encode())
                if n < 0:
                    raise RuntimeError(f"axon_stop_nrt_profile rc={n}")

        return _hook

    hook = _mk(so_path)
    mod = types.ModuleType("antenv.axon_hooks")
    mod.get_axon_ntff_profile_hook = lambda: hook
    mod.set_axon_ntff_profile_hook = lambda h: None
    sys.modules["antenv.axon_hooks"] = mod
    antenv.axon_hooks = mod


# ---------------------------------------------------------------------------
# Kernel
# ---------------------------------------------------------------------------
B, SEQ, D = 4, 4096, 1024
HEADS, DH = 8, 64
INNER = HEADS * DH  # 512
NCORES = 8
NTOK = B * SEQ // NCORES  # 2048 tokens per core
NT = NTOK // 128  # 16
CK = D // 128  # 8 contraction chunks for the qkv projection
KC = INNER // 128  # 4 contraction chunks for the output projection
REPLICA_GROUPS = [[0, 1], [2, 3], [4, 5], [6, 7]]

_BUILT = None
LAST_RESULT = {}


def build_kernel(debug: bool = False):
    import concourse.bass as bass
    import concourse.mybir as mybir
    import concourse.tile as tile

    BF = mybir.dt.bfloat16
    F32 = mybir.dt.float32
    EXP = mybir.ActivationFunctionType.Exp
    COPY = mybir.ActivationFunctionType.Copy
    X = mybir.AxisListType.X

    nc = bass.Bass(name="linattn")
    xT = nc.declare_dram_parameter("xT", [D, NTOK], BF, isOutput=False)
    wqT = nc.declare_dram_parameter("wqT", [D, INNER], BF, isOutput=False)
    wkvT = nc.declare_dram_parameter("wkvT", [D, 2 * INNER], BF, isOutput=False)
    woutT = nc.declare_dram_parameter("woutT", [INNER, D], BF, isOutput=False)
    bias = nc.declare_dram_parameter("bias", [128, D], F32, isOutput=False)
    y = nc.declare_dram_parameter("y", [NTOK, D], F32, isOutput=True)
    if debug:
        ct_dbg = nc.declare_dram_parameter("ct_dbg", [DH, INNER], F32, isOutput=True)
        ctr_dbg = nc.declare_dram_parameter(
            "ctr_dbg", [128, KC, DH], F32, isOutput=True
        )
        g_dbg = nc.declare_dram_parameter("g_dbg", [128, KC, D], F32, isOutput=True)
        qt_dbg = nc.declare_dram_parameter(
            "qt_dbg", [128, KC, NTOK], F32, isOutput=True
        )
        kv_dbg = nc.declare_dram_parameter("kv_dbg", [128, 2 * INNER], F32, isOutput=True)

    with contextlib.ExitStack() as ctx:
        tc = ctx.enter_context(tile.TileContext(nc))
        cpool = ctx.enter_context(tc.tile_pool(name="const", bufs=1))
        wpool = ctx.enter_context(tc.tile_pool(name="work", bufs=4))
        opool = ctx.enter_context(tc.tile_pool(name="yout", bufs=3))
        dpool = ctx.enter_context(tc.tile_pool(name="dram", bufs=1, space="DRAM"))

        # ---- resident loads ------------------------------------------------
        # split per contraction chunk so the first matmuls start as soon as
        # their own slices land
        # spread issue across engine queues: the sequencers trigger DMAs at
        # ~0.65us each, so a single queue serializes the input loads
        wkvT_r = wkvT.rearrange("(ck p) f -> p ck f", p=128)
        xT_r = xT.rearrange("(ck p) n -> p ck n", p=128)
        wkv_t = [cpool.tile([128, 2 * INNER], BF, name=f"wkv_t{ck}") for ck in range(CK)]
        x_t = [cpool.tile([128, NTOK], BF, name=f"x_t{ck}") for ck in range(CK)]
        for ck in range(CK):
            nc.scalar.dma_start(wkv_t[ck][:], wkvT_r[:, ck])
            nc.sync.dma_start(x_t[ck][:, :512], xT_r[:, ck, :512])
        for q4 in range(1, 4):
            sl = slice(q4 * 512, (q4 + 1) * 512)
            for ck in range(CK):
                nc.gpsimd.dma_start(x_t[ck][:, sl], xT_r[:, ck, sl])
        wq_s = cpool.tile([128, CK, INNER], BF, name="wq_s")
        wout_s = cpool.tile([128, KC, D], BF, name="wout_s")
        bias_s = cpool.tile([128, D], F32, name="bias_s")

        qT_s = cpool.tile([128, KC, NTOK], BF, name="qT_s")
        g_s = cpool.tile([128, KC, D], BF, name="g_s")
        ct_f = [
            cpool.tile([DH, HEADS * DH], F32, name=f"ct_f{i}") for i in range(2)
        ]
        ct_r = cpool.tile([128, KC, DH], F32, name="ct_r")
        # block-diagonal bf16 form: [e-half, pair, half, d]
        ctw = cpool.tile([128, KC, 2, DH], BF, name="ctw")
        nc.vector.memset(ctw[:], 0.0)

        # ---- phase B: k,v projection + softmax + CT partial ---------------
        # CT[e, h*64+d] = sum_n v[n,e]/s[n,h] * exp(k[n,d])   (this core's n)
        # Two n-halves, each followed by its own all-reduce: the first
        # collective also acts as a rendezvous between the core pair, so the
        # second one (the one on the critical path) runs with minimal skew
        # and hides behind the q projection.
        ps_ct_cm = tc.tile_pool(name="ps_ct", bufs=2, space="PSUM")
        ps_ct = ps_ct_cm.__enter__()
        ps_kv_cm = tc.tile_pool(name="ps_kv", bufs=3, space="PSUM")
        ps_kv = ps_kv_cm.__enter__()
        HNT = NT // 2
        cin = [dpool.tile([128, KC, DH], F32, name=f"cc_in{i}") for i in range(2)]
        cout = [dpool.tile([128, KC, DH], F32, name=f"cc_out{i}") for i in range(2)]
        for stage in range(2):
            ct_ps = ps_ct.tile([DH, HEADS * DH], F32, name="ct_ps")
            for nt in range(stage * HNT, (stage + 1) * HNT):
                kv_ps = ps_kv.tile([128, 2 * INNER], F32, name="kv_ps")
                nsl = slice(nt * 128, (nt + 1) * 128)
                for ck in range(CK):
                    nc.tensor.matmul(
                        kv_ps[:, :INNER],
                        lhsT=x_t[ck][:, nsl],
                        rhs=wkv_t[ck][:, :INNER],
                        start=(ck == 0),
                        stop=(ck == CK - 1),
                    )
                    nc.tensor.matmul(
                        kv_ps[:, INNER:],
                        lhsT=x_t[ck][:, nsl],
                        rhs=wkv_t[ck][:, INNER:],
                        start=(ck == 0),
                        stop=(ck == CK - 1),
                    )
                expk = wpool.tile([128, INNER], BF, name="expk")
                nc.scalar.activation(expk[:], kv_ps[:, :INNER], EXP)
                ssum = wpool.tile([128, HEADS], F32, name="ssum")
                nc.vector.reduce_sum(
                    ssum[:], expk.rearrange("p (h d) -> p h d", d=DH), axis=X
                )
                rec = wpool.tile([128, HEADS], F32, name="rec")
                nc.vector.reciprocal(rec[:], ssum[:])
                vsc = wpool.tile([128, INNER], BF, name="vsc")
                nc.vector.tensor_tensor(
                    vsc.rearrange("p (h d) -> p h d", d=DH),
                    kv_ps[:, INNER:].rearrange("p (h d) -> p h d", d=DH),
                    rec[:, :, None].to_broadcast([128, HEADS, DH]),
                    mybir.AluOpType.mult,
                )
                for h in range(HEADS):
                    hs = slice(h * DH, (h + 1) * DH)
                    # start=True clears the whole PSUM *bank*, so only the
                    # very first matmul of the bank may set it; later heads'
                    # first write lands on has_written=0 elems -> overwrite.
                    nc.tensor.matmul(
                        ct_ps[:, hs],
                        lhsT=vsc[:, hs],
                        rhs=expk[:, hs],
                        start=(nt == stage * HNT and h == 0),
                        stop=(nt == (stage + 1) * HNT - 1),
                        skip_group_check=True,
                    )
                if debug and nt == 0:
                    kvtmp = cpool.tile([128, 2 * INNER], F32, name="kvtmp")
                    nc.vector.tensor_copy(kvtmp[:], kv_ps[:])
                    nc.sync.dma_start(kv_dbg[:], kvtmp[:])
            nc.vector.tensor_copy(ct_f[stage][:], ct_ps[:])
            # pack even heads on partitions 0-63, odd heads on 64-127
            ct_v = ct_f[stage].rearrange("e (pr two d) -> e pr two d", two=2, d=DH)
            nc.gpsimd.dma_start(cin[stage][:DH], ct_v[:, :, 0, :])
            nc.gpsimd.dma_start(cin[stage][DH:], ct_v[:, :, 1, :])
            nc.gpsimd.collective_compute(
                "AllReduce",
                mybir.AluOpType.add,
                replica_groups=REPLICA_GROUPS,
                ins=[cin[stage].opt()],
                outs=[cout[stage].opt()],
            )
            if stage == 0:
                # queue the later-phase weight loads behind the x/wkv pieces
                nc.sync.dma_start(
                    wq_s[:], wqT.rearrange("(ck p) f -> p ck f", p=128)
                )
                nc.sync.dma_start(
                    wout_s[:], woutT.rearrange("(kc p) f -> p kc f", p=128)
                )
                nc.sync.dma_start(bias_s[:], bias[:])
        ps_kv_cm.__exit__(None, None, None)
        ps_ct_cm.__exit__(None, None, None)

        ct_h0 = cpool.tile([128, KC, DH], F32, name="ct_h0")
        ct_h1 = cpool.tile([128, KC, DH], F32, name="ct_h1")
        nc.scalar.dma_start(ct_h0[:], cout[0][:])
        nc.scalar.dma_start(ct_h1[:], cout[1][:])
        nc.vector.tensor_add(ct_r[:], ct_h0[:], ct_h1[:])
        nc.vector.tensor_copy(ctw[:DH, :, 0, :], ct_r[:DH])
        nc.vector.tensor_copy(ctw[DH:, :, 1, :], ct_r[DH:])
        if debug:
            nc.sync.dma_start(ct_dbg[:], ct_f[1][:])
            nc.sync.dma_start(ctr_dbg[:], ct_r[:])

        # ---- phase C: q^T projection (overlaps the collective) ------------
        ps_q_cm = tc.tile_pool(name="ps_q", bufs=6, space="PSUM")
        ps_q = ps_q_cm.__enter__()
        for i in range(KC):
            q_ps = [ps_q.tile([128, 512], F32, name="q_ps") for _ in range(4)]
            wq_i = wq_s[:, :, i * 128 : (i + 1) * 128]
            for ck in range(CK):
                for ntile in range(4):
                    nc.tensor.matmul(
                        q_ps[ntile][:],
                        lhsT=wq_i[:, ck],
                        rhs=x_t[ck][:, ntile * 512 : (ntile + 1) * 512],
                        start=(ck == 0),
                        stop=(ck == CK - 1),
                    )
            for ntile in range(4):
                nc.scalar.activation(
                    qT_s[:, i, ntile * 512 : (ntile + 1) * 512], q_ps[ntile][:], COPY
                )

        ps_q_cm.__exit__(None, None, None)

        # ---- phase D: G = blockdiag(context^T) @ w_out^T -------------------
        ps_g = ctx.enter_context(tc.tile_pool(name="ps_g", bufs=2, space="PSUM"))
        ps_y = ctx.enter_context(tc.tile_pool(name="ps_y", bufs=2, space="PSUM"))
        for pr in range(KC):
            g_ps = ps_g.tile([128, D], F32, name="g_ps")
            lhs = ctw[:, pr].rearrange("p two d -> p (two d)")
            nc.tensor.matmul(
                g_ps[:, :512], lhsT=lhs, rhs=wout_s[:, pr, :512], start=True, stop=True
            )
            nc.tensor.matmul(
                g_ps[:, 512:], lhsT=lhs, rhs=wout_s[:, pr, 512:], start=True, stop=True
            )
            nc.scalar.activation(g_s[:, pr, :], g_ps[:], COPY)
        if debug:
            gtmp = cpool.tile([128, KC, D], F32, name="gtmp")
            nc.vector.tensor_copy(gtmp[:], g_s[:])
            nc.sync.dma_start(g_dbg[:], gtmp[:])
            qtmp = cpool.tile([128, KC, NTOK], F32, name="qtmp")
            nc.vector.tensor_copy(qtmp[:], qT_s[:])
            nc.sync.dma_start(qt_dbg[:], qtmp[:])

        # ---- phase E: y = q @ G + b ----------------------------------------
        for nt in range(NT):
            y_ps = ps_y.tile([128, D], F32, name="y_ps")
            for kc in range(KC):
                q_nt = qT_s[:, kc, nt * 128 : (nt + 1) * 128]
                nc.tensor.matmul(
                    y_ps[:, :512],
                    lhsT=q_nt,
                    rhs=g_s[:, kc, :512],
                    start=(kc == 0),
                    stop=(kc == KC - 1),
                )
                nc.tensor.matmul(
                    y_ps[:, 512:],
                    lhsT=q_nt,
                    rhs=g_s[:, kc, 512:],
                    start=(kc == 0),
                    stop=(kc == KC - 1),
                )
            y_t = opool.tile([128, D], F32, name="y_t")
            nc.vector.tensor_add(y_t[:], y_ps[:], bias_s[:])
            nc.sync.dma_start(y[nt * 128 : (nt + 1) * 128, :], y_t[:])

    return nc


def _prep_inputs(x, w_qkv, w_out, b_out):
    bf16 = ml_dtypes.bfloat16
    x = np.asarray(x, dtype=np.float32)
    w_qkv = np.asarray(w_qkv, dtype=np.float32)
    w_out = np.asarray(w_out, dtype=np.float32)
    b_out = np.asarray(b_out, dtype=np.float32)

    wqT = np.ascontiguousarray(w_qkv[:INNER].T).astype(bf16)  # [D, 512]
    wkvT = np.ascontiguousarray(w_qkv[INNER:].T).astype(bf16)  # [D, 1024]
    woutT = np.ascontiguousarray(w_out.T).astype(bf16)  # [512, D]
    bias = np.ascontiguousarray(np.broadcast_to(b_out[None, :], (128, D))).astype(
        np.float32
    )
    xs = x.reshape(B, 2, NTOK, D)
    in_maps = []
    for c in range(NCORES):
        xT = np.ascontiguousarray(xs[c // 2, c % 2].T).astype(bf16)  # [D, NTOK]
        in_maps.append(
            {"xT": xT, "wqT": wqT, "wkvT": wkvT, "woutT": woutT, "bias": bias}
        )
    return in_maps


def kernel(x, w_qkv, w_out, b_out):
    global _BUILT
    _install_compile_patch()
    if os.environ.get("BASS_TRACE"):
        _install_ntff_hook()
    from concourse.bass_utils import run_bass_kernel_spmd

    if _BUILT is None:
        _BUILT = build_kernel()
    nc = _BUILT
    in_maps = _prep_inputs(x, w_qkv, w_out, b_out)
    res = run_bass_kernel_spmd(nc, in_maps, core_ids=list(range(NCORES)))
    LAST_RESULT["exec_time_ns"] = res.exec_time_ns
    LAST_RESULT["profile_json"] = res.profile_json
    out = np.empty((B, 2, NTOK, D), dtype=np.float32)
    for c in range(NCORES):
        out[c // 2, c % 2] = res.results[c]["y"]
    return out.reshape(B, SEQ, D)


# revision 27
# speedup vs baseline: 1.0675x; 1.0675x over previous
"""LinearAttention kernel for one TRN2 chip (8 NeuronCores), Bass/Tile.

Math (per batch b):
  qkv = x @ w_qkv.T ; q,k,v split, per-head [n, 64]
  k_s = softmax(k, axis=-1)              (over dh, per token/head)
  context_h = k_s^T @ v                  [64, 64]
  out_h = q_h @ context_h ; y = out @ w_out.T + b

Restructured as:
  CT_h = (v/s)^T-style partial:  CT[e,d] = sum_n v[n,e]/s[n,h] * exp(k[n,d])
  G_h  = context_h @ w_out_h^T   -> G [inner=512, 1024] block rows
  y    = q @ G + b               (single K=512 matmul)

Sharding: 8 shards = (batch, half-sequence); each core computes its
2048 tokens end-to-end; only the tiny per-batch context (128 KiB) is
all-reduced between the two cores sharing a batch.

Device layouts avoid all transposes on-chip: the host feeds x^T, w^T.
Compute dtype bf16 (fp32 PSUM accumulation), output fp32.
"""

import contextlib
import ctypes
import os
import sys
import types

import numpy as np
import ml_dtypes

# ---------------------------------------------------------------------------
# Compat shim 1: the walrus in this image supports only ONE semaphore wait
# per instruction; split multi-wait instructions into prefix NoOps.
# ---------------------------------------------------------------------------
_MAX_WAITS = 1


def _legalize_bir(bir_bytes: bytes) -> bytes:
    import orjson

    bir = orjson.loads(bir_bytes)
    changed = False
    for fn in bir.get("functions", []):
        for blk in fn.get("blocks", []):
            new_insts = []
            for ins in blk.get("instructions", []):
                si = ins.get("sync_info") or {}
                waits = si.get("on_wait") or []
                if len(waits) > _MAX_WAITS:
                    changed = True
                    extra, keep = waits[:-_MAX_WAITS], waits[-_MAX_WAITS:]
                    for i in range(0, len(extra), _MAX_WAITS):
                        new_insts.append(
                            {
                                "name": f"{ins['name']}-ws{i}",
                                "opcode": "NoOp",
                                "engine": ins["engine"],
                                "ins": [],
                                "outs": [],
                                "sync_info": {
                                    "on_update": [],
                                    "on_wait": extra[i : i + _MAX_WAITS],
                                },
                            }
                        )
                    si["on_wait"] = keep
                new_insts.append(ins)
            blk["instructions"] = new_insts
    if not changed:
        return bir_bytes
    return orjson.dumps(bir)


_compile_patched = False


def _install_compile_patch():
    global _compile_patched
    if _compile_patched:
        return
    import concourse.bass2jax as bass2jax
    import concourse.bass_utils as bass_utils

    orig = bass2jax.compile_bir_kernel

    def compile_bir_kernel_legalized(bir_json, tmpdir, neff_name="file.neff"):
        return orig(_legalize_bir(bytes(bir_json)), tmpdir, neff_name=neff_name)

    bass2jax.compile_bir_kernel = compile_bir_kernel_legalized

    if os.environ.get("LINATTN_LDW_OPT"):
        orig_run = bass_utils.run_command

        def run_command_ldwopt(argv, **kw):
            argv = [
                "--enable-ldw-opt=true" if a == "--enable-ldw-opt=false" else a
                for a in argv
            ]
            return orig_run(argv, **kw)

        bass_utils.run_command = run_command_ldwopt
    _compile_patched = True


# ---------------------------------------------------------------------------
# Compat shim 2: NTFF profiling hook (only needed when BASS_TRACE is set).
# ---------------------------------------------------------------------------
def _install_ntff_hook():
    import antenv

    if "antenv.axon_hooks" in sys.modules:
        return
    so_path = "/opt/axon/libaxon_pjrt.so"

    def _mk(so_path):
        try:
            lib = ctypes.CDLL(so_path)
        except OSError:
            return None
        if not hasattr(lib, "axon_start_nrt_profile"):
            return None
        lib.axon_start_nrt_profile.argtypes = [
            ctypes.POINTER(ctypes.c_int64),
            ctypes.c_size_t,
        ]
        lib.axon_start_nrt_profile.restype = ctypes.c_int64
        lib.axon_stop_nrt_profile.argtypes = [ctypes.c_char_p]
        lib.axon_stop_nrt_profile.restype = ctypes.c_int64

        @contextlib.contextmanager
        def _hook(output_dir, device_ids):
            import jax

            jax.devices()
            if device_ids:
                ids = (ctypes.c_int64 * len(device_ids))(*device_ids)
                rc = lib.axon_start_nrt_profile(ids, len(device_ids))
            else:
                rc = lib.axon_start_nrt_profile(None, 0)
            if rc != 0:
                raise RuntimeError(f"axon_start_nrt_profile rc={rc}")
            try:
                yield
            finally:
                n = lib.axon_stop_nrt_profile(str(output_dir).encode())
                if n < 0:
                    raise RuntimeError(f"axon_stop_nrt_profile rc={n}")

        return _hook

    hook = _mk(so_path)
    mod = types.ModuleType("antenv.axon_hooks")
    mod.get_axon_ntff_profile_hook = lambda: hook
    mod.set_axon_ntff_profile_hook = lambda h: None
    sys.modules["antenv.axon_hooks"] = mod
    antenv.axon_hooks = mod


# ---------------------------------------------------------------------------
# Kernel
# ---------------------------------------------------------------------------
B, SEQ, D = 4, 4096, 1024
HEADS, DH = 8, 64
INNER = HEADS * DH  # 512
NCORES = 8
NTOK = B * SEQ // NCORES  # 2048 tokens per core
NT = NTOK // 128  # 16
CK = D // 128  # 8 contraction chunks for the qkv projection
KC = INNER // 128  # 4 contraction chunks for the output projection
REPLICA_GROUPS = [[0, 1], [2, 3], [4, 5], [6, 7]]

_BUILT = None
LAST_RESULT = {}


def build_kernel(debug: bool = False):
    import concourse.bass as bass
    import concourse.mybir as mybir
    import concourse.tile as tile

    BF = mybir.dt.bfloat16
    F32 = mybir.dt.float32
    EXP = mybir.ActivationFunctionType.Exp
    COPY = mybir.ActivationFunctionType.Copy
    X = mybir.AxisListType.X

    nc = bass.Bass(name="linattn")
    xT = nc.declare_dram_parameter("xT", [D, NTOK], BF, isOutput=False)
    wqT = nc.declare_dram_parameter("wqT", [D, INNER], BF, isOutput=False)
    wkvT = nc.declare_dram_parameter("wkvT", [D, 2 * INNER], BF, isOutput=False)
    woutT = nc.declare_dram_parameter("woutT", [INNER, D], BF, isOutput=False)
    bias = nc.declare_dram_parameter("bias", [128, D], F32, isOutput=False)
    y = nc.declare_dram_parameter("y", [NTOK, D], F32, isOutput=True)
    if debug:
        ct_dbg = nc.declare_dram_parameter("ct_dbg", [DH, INNER], F32, isOutput=True)
        ctr_dbg = nc.declare_dram_parameter(
            "ctr_dbg", [128, KC, DH], F32, isOutput=True
        )
        g_dbg = nc.declare_dram_parameter("g_dbg", [128, KC, D], F32, isOutput=True)
        qt_dbg = nc.declare_dram_parameter(
            "qt_dbg", [128, KC, NTOK], F32, isOutput=True
        )
        kv_dbg = nc.declare_dram_parameter("kv_dbg", [128, 2 * INNER], F32, isOutput=True)

    with contextlib.ExitStack() as ctx:
        tc = ctx.enter_context(tile.TileContext(nc))
        cpool = ctx.enter_context(tc.tile_pool(name="const", bufs=1))
        wpool = ctx.enter_context(tc.tile_pool(name="work", bufs=4))
        opool = ctx.enter_context(tc.tile_pool(name="yout", bufs=3))
        dpool = ctx.enter_context(tc.tile_pool(name="dram", bufs=1, space="DRAM"))

        # ---- resident loads ------------------------------------------------
        # split per contraction chunk so the first matmuls start as soon as
        # their own slices land
        wkvT_r = wkvT.rearrange("(ck p) f -> p ck f", p=128)
        xT_r = xT.rearrange("(ck p) n -> p ck n", p=128)
        wkv_t = [cpool.tile([128, 2 * INNER], BF, name=f"wkv_t{ck}") for ck in range(CK)]
        x_t = [cpool.tile([128, NTOK], BF, name=f"x_t{ck}") for ck in range(CK)]
        # first-quarter x and wkv slices land first so nt=0 starts ASAP
        for ck in range(CK):
            nc.sync.dma_start(wkv_t[ck][:, :INNER], wkvT_r[:, ck, :INNER])
            nc.sync.dma_start(x_t[ck][:, :512], xT_r[:, ck, :512])
            nc.sync.dma_start(wkv_t[ck][:, INNER:], wkvT_r[:, ck, INNER:])
        for q4 in range(1, 4):
            sl = slice(q4 * 512, (q4 + 1) * 512)
            for ck in range(CK):
                nc.sync.dma_start(x_t[ck][:, sl], xT_r[:, ck, sl])
        wq_s = cpool.tile([128, CK, INNER], BF, name="wq_s")
        wout_s = cpool.tile([128, KC, D], BF, name="wout_s")
        bias_s = cpool.tile([128, D], F32, name="bias_s")

        qT_s = cpool.tile([128, KC, NTOK], BF, name="qT_s")
        g_s = cpool.tile([128, KC, D], BF, name="g_s")
        ct_f = [
            cpool.tile([DH, HEADS * DH], F32, name=f"ct_f{i}") for i in range(2)
        ]
        ct_r = cpool.tile([128, KC, DH], F32, name="ct_r")
        # block-diagonal bf16 form: [e-half, pair, half, d]
        ctw = cpool.tile([128, KC, 2, DH], BF, name="ctw")
        nc.vector.memset(ctw[:], 0.0)

        # ---- phase B: k,v projection + softmax + CT partial ---------------
        # CT[e, h*64+d] = sum_n v[n,e]/s[n,h] * exp(k[n,d])   (this core's n)
        # Two n-halves, each followed by its own all-reduce: the first
        # collective also acts as a rendezvous between the core pair, so the
        # second one (the one on the critical path) runs with minimal skew
        # and hides behind the q projection.
        ps_ct_cm = tc.tile_pool(name="ps_ct", bufs=2, space="PSUM")
        ps_ct = ps_ct_cm.__enter__()
        ps_kv_cm = tc.tile_pool(name="ps_kv", bufs=3, space="PSUM")
        ps_kv = ps_kv_cm.__enter__()
        HNT = NT // 2
        cin = [dpool.tile([128, KC, DH], F32, name=f"cc_in{i}") for i in range(2)]
        cout = [dpool.tile([128, KC, DH], F32, name=f"cc_out{i}") for i in range(2)]
        for stage in range(2):
            ct_ps = ps_ct.tile([DH, HEADS * DH], F32, name="ct_ps")
            for nt in range(stage * HNT, (stage + 1) * HNT):
                kv_ps = ps_kv.tile([128, 2 * INNER], F32, name="kv_ps")
                nsl = slice(nt * 128, (nt + 1) * 128)
                for ck in range(CK):
                    nc.tensor.matmul(
                        kv_ps[:, :INNER],
                        lhsT=x_t[ck][:, nsl],
                        rhs=wkv_t[ck][:, :INNER],
                        start=(ck == 0),
                        stop=(ck == CK - 1),
                    )
                    nc.tensor.matmul(
                        kv_ps[:, INNER:],
                        lhsT=x_t[ck][:, nsl],
                        rhs=wkv_t[ck][:, INNER:],
                        start=(ck == 0),
                        stop=(ck == CK - 1),
                    )
                expk = wpool.tile([128, INNER], BF, name="expk")
                nc.scalar.activation(expk[:], kv_ps[:, :INNER], EXP)
                ssum = wpool.tile([128, HEADS], F32, name="ssum")
                nc.vector.reduce_sum(
                    ssum[:], expk.rearrange("p (h d) -> p h d", d=DH), axis=X
                )
                rec = wpool.tile([128, HEADS], F32, name="rec")
                nc.vector.reciprocal(rec[:], ssum[:])
                vsc = wpool.tile([128, INNER], BF, name="vsc")
                nc.vector.tensor_tensor(
                    vsc.rearrange("p (h d) -> p h d", d=DH),
                    kv_ps[:, INNER:].rearrange("p (h d) -> p h d", d=DH),
                    rec[:, :, None].to_broadcast([128, HEADS, DH]),
                    mybir.AluOpType.mult,
                )
                for h in range(HEADS):
                    hs = slice(h * DH, (h + 1) * DH)
                    # start=True clears the whole PSUM *bank*, so only the
                    # very first matmul of the bank may set it; later heads'
                    # first write lands on has_written=0 elems -> overwrite.
                    nc.tensor.matmul(
                        ct_ps[:, hs],
                        lhsT=vsc[:, hs],
                        rhs=expk[:, hs],
                        start=(nt == stage * HNT and h == 0),
                        stop=(nt == (stage + 1) * HNT - 1),
                        skip_group_check=True,
                    )
                if debug and nt == 0:
                    kvtmp = cpool.tile([128, 2 * INNER], F32, name="kvtmp")
                    nc.vector.tensor_copy(kvtmp[:], kv_ps[:])
                    nc.sync.dma_start(kv_dbg[:], kvtmp[:])
            nc.vector.tensor_copy(ct_f[stage][:], ct_ps[:])
            # pack even heads on partitions 0-63, odd heads on 64-127
            ct_v = ct_f[stage].rearrange("e (pr two d) -> e pr two d", two=2, d=DH)
            nc.gpsimd.dma_start(cin[stage][:DH], ct_v[:, :, 0, :])
            nc.gpsimd.dma_start(cin[stage][DH:], ct_v[:, :, 1, :])
            nc.gpsimd.collective_compute(
                "AllReduce",
                mybir.AluOpType.add,
                replica_groups=REPLICA_GROUPS,
                ins=[cin[stage].opt()],
                outs=[cout[stage].opt()],
            )
            if stage == 0:
                # queue the later-phase weight loads behind the x/wkv pieces
                nc.sync.dma_start(
                    wq_s[:], wqT.rearrange("(ck p) f -> p ck f", p=128)
                )
                nc.sync.dma_start(
                    wout_s[:], woutT.rearrange("(kc p) f -> p kc f", p=128)
                )
                nc.sync.dma_start(bias_s[:], bias[:])
        ps_kv_cm.__exit__(None, None, None)
        ps_ct_cm.__exit__(None, None, None)

        ct_h0 = cpool.tile([128, KC, DH], F32, name="ct_h0")
        ct_h1 = cpool.tile([128, KC, DH], F32, name="ct_h1")
        nc.scalar.dma_start(ct_h0[:], cout[0][:])
        nc.scalar.dma_start(ct_h1[:], cout[1][:])
        nc.vector.tensor_add(ct_r[:], ct_h0[:], ct_h1[:])
        nc.vector.tensor_copy(ctw[:DH, :, 0, :], ct_r[:DH])
        nc.vector.tensor_copy(ctw[DH:, :, 1, :], ct_r[DH:])
        if debug:
            nc.sync.dma_start(ct_dbg[:], ct_f[1][:])
            nc.sync.dma_start(ctr_dbg[:], ct_r[:])

        # ---- phase C: q^T projection (overlaps the collective) ------------
        ps_q_cm = tc.tile_pool(name="ps_q", bufs=6, space="PSUM")
        ps_q = ps_q_cm.__enter__()
        for i in range(KC):
            q_ps = [ps_q.tile([128, 512], F32, name="q_ps") for _ in range(4)]
            wq_i = wq_s[:, :, i * 128 : (i + 1) * 128]
            for ck in range(CK):
                for ntile in range(4):
                    nc.tensor.matmul(
                        q_ps[ntile][:],
                        lhsT=wq_i[:, ck],
                        rhs=x_t[ck][:, ntile * 512 : (ntile + 1) * 512],
                        start=(ck == 0),
                        stop=(ck == CK - 1),
                    )
            for ntile in range(4):
                nc.scalar.activation(
                    qT_s[:, i, ntile * 512 : (ntile + 1) * 512], q_ps[ntile][:], COPY
                )

        ps_q_cm.__exit__(None, None, None)

        # ---- phase D: G = blockdiag(context^T) @ w_out^T -------------------
        ps_g = ctx.enter_context(tc.tile_pool(name="ps_g", bufs=2, space="PSUM"))
        ps_y = ctx.enter_context(tc.tile_pool(name="ps_y", bufs=2, space="PSUM"))
        for pr in range(KC):
            g_ps = ps_g.tile([128, D], F32, name="g_ps")
            lhs = ctw[:, pr].rearrange("p two d -> p (two d)")
            nc.tensor.matmul(
                g_ps[:, :512], lhsT=lhs, rhs=wout_s[:, pr, :512], start=True, stop=True
            )
            nc.tensor.matmul(
                g_ps[:, 512:], lhsT=lhs, rhs=wout_s[:, pr, 512:], start=True, stop=True
            )
            nc.scalar.activation(g_s[:, pr, :], g_ps[:], COPY)
        if debug:
            gtmp = cpool.tile([128, KC, D], F32, name="gtmp")
            nc.vector.tensor_copy(gtmp[:], g_s[:])
            nc.sync.dma_start(g_dbg[:], gtmp[:])
            qtmp = cpool.tile([128, KC, NTOK], F32, name="qtmp")
            nc.vector.tensor_copy(qtmp[:], qT_s[:])
            nc.sync.dma_start(qt_dbg[:], qtmp[:])

        # ---- phase E: y = q @ G + b ----------------------------------------
        for nt in range(NT):
            y_ps = ps_y.tile([128, D], F32, name="y_ps")
            for kc in range(KC):
                q_nt = qT_s[:, kc, nt * 128 : (nt + 1) * 128]
                nc.tensor.matmul(
                    y_ps[:, :512],
                    lhsT=q_nt,
                    rhs=g_s[:, kc, :512],
                    start=(kc == 0),
                    stop=(kc == KC - 1),
                )
                nc.tensor.matmul(
                    y_ps[:, 512:],
                    lhsT=q_nt,
                    rhs=g_s[:, kc, 512:],
                    start=(kc == 0),
                    stop=(kc == KC - 1),
                )
            y_t = opool.tile([128, D], F32, name="y_t")
            nc.vector.tensor_add(y_t[:], y_ps[:], bias_s[:])
            nc.sync.dma_start(y[nt * 128 : (nt + 1) * 128, :], y_t[:])

    return nc


def _prep_inputs(x, w_qkv, w_out, b_out):
    bf16 = ml_dtypes.bfloat16
    x = np.asarray(x, dtype=np.float32)
    w_qkv = np.asarray(w_qkv, dtype=np.float32)
    w_out = np.asarray(w_out, dtype=np.float32)
    b_out = np.asarray(b_out, dtype=np.float32)

    wqT = np.ascontiguousarray(w_qkv[:INNER].T).astype(bf16)  # [D, 512]
    wkvT = np.ascontiguousarray(w_qkv[INNER:].T).astype(bf16)  # [D, 1024]
    woutT = np.ascontiguousarray(w_out.T).astype(bf16)  # [512, D]
    bias = np.ascontiguousarray(np.broadcast_to(b_out[None, :], (128, D))).astype(
        np.float32
    )
    xs = x.reshape(B, 2, NTOK, D)
    in_maps = []
    for c in range(NCORES):
        xT = np.ascontiguousarray(xs[c // 2, c % 2].T).astype(bf16)  # [D, NTOK]
        in_maps.append(
            {"xT": xT, "wqT": wqT, "wkvT": wkvT, "woutT": woutT, "bias": bias}
        )
    return in_maps


def kernel(x, w_qkv, w_out, b_out):
    global _BUILT
    _install_compile_patch()
    if os.environ.get("BASS_TRACE"):
        _install_ntff_hook()
    from concourse.bass_utils import run_bass_kernel_spmd

    if _BUILT is None:
        _BUILT = build_kernel()
    nc = _BUILT
    in_maps = _prep_inputs(x, w_qkv, w_out, b_out)
    res = run_bass_kernel_spmd(nc, in_maps, core_ids=list(range(NCORES)))
    LAST_RESULT["exec_time_ns"] = res.exec_time_ns
    LAST_RESULT["profile_json"] = res.profile_json
    out = np.empty((B, 2, NTOK, D), dtype=np.float32)
    for c in range(NCORES):
        out[c // 2, c % 2] = res.results[c]["y"]
    return out.reshape(B, SEQ, D)
